# revision 69
# baseline (speedup 1.0000x reference)
"""LCSA (local convolutional sparse attention) Trainium2 Bass kernel.

Problem: B=2, S=2048, D=1024, H=8 heads, E=128 head width, KW=16 kernel width,
per-head dilations [1,1,2,2,4,4,8,8].

Sharding: pure data-parallel over (batch, sequence): core c handles batch c//4,
sequence chunk (c%4)*512..+512. Each core loads a 640-token haloed slice of x
(64-token halo each side, zero-padded at batch edges; padding reproduces the
reference's "invalid position -> bias" semantics exactly since k(0)=bk, v(0)=bv).

Device algorithm per core (f32r q/k path for accuracy, bf16 value path,
fp32 PSUM; measured rel err 4.2e-3):
  - Host packs Wq/Wk (f32) and Wv (bf16) as [D, H*E] so every DMA descriptor
    is >=512B (full-rate DMA); x ships twice: f32 (q/k) + bf16 (v). DMA order
    feeds the v projection first (cheap bf16 operands) so PE starts ~3us in
    while the fat f32 q/k weights stream behind; ~20 dummy matmuls at t~1us
    ramp the PE clock to full speed before real work arrives.
  - v = xT.T @ Wv [640, H*E] (bf16, xT chunks stationary); then qT[h] =
    Wq[h].T @ xT [E,512] and kT[h] in f32r (1 cyc/row since all free dims
    >=256; per-head dilation-tailored k spans skip the permanently-masked
    halo columns, edges zeroed once, spans 4-aligned for f32r).
  - Attention runs as one flat 32-chain software pipeline (c=(i,h), stage
    offsets F+6/T+4/A+3 ahead of O), with the F-prologue hidden under the
    last two qk heads:
    F: logits PSUM accumulates mask[h] via an fp8e5 DoubleRow matmul
       (0.5 cyc/row; -28672 mask exact in e5m2) then qT_tile.T @ kT_window
       -> [128,256]; ACT exp straight off PSUM (no max subtraction:
       |logit| <~ 60 is safe in fp32) with accumulated row-sum; Pool
       normalize_recip divides by the row-sum -> bf16 scores.
    T: PE transposes scores (bf16 identity, 1 cyc/row); DVE copies to SBUF.
    A: attnT = v.T-chunks @ scoreT (bf16); DVE copies to SBUF bf16.
    O: out[i] += attnT[h].T @ Wo[h] (bf16, Wo pre-scaled by E**-0.5); out
       quarters copied on ACT+DVE in parallel and DMA'd per half.
  - PSUM banks: pj/ou shared pool 3 + lg 2 + st 1 + at 2 = 8.
  - Biases are all zero in this problem; kernel() dispatches to a generic
    biased variant if any bias is nonzero.
"""

import numpy as np

B, S, D, H, E, KW = 2, 2048, 1024, 8, 128, 16
HALO = 64          # covers max offset d*(KW-1)//2 = 60 for d=8
CHUNK = 512        # query tokens per core
SPAN = CHUNK + 2 * HALO   # 640 = 5*128 kv tokens per core
NST = SPAN // 128  # 5 sequence tiles
NQT = CHUNK // 128 # 4 query tiles
NC_ = 8            # cores
DC = D // 128      # 8 contraction chunks
MASKVAL = -30000.0
DILS = [1, 1, 2, 2, 4, 4, 8, 8]
# per-head needed kT span [lo, lo+w), aligned to 4 for f32r matmul APs
_KLO = [((HALO - (d * (KW - 1)) // 2) // 4) * 4 for d in DILS]
_KW_ = [-(-(HALO - (d * (KW - 1)) // 2 + CHUNK + (KW - 1) * d - lo) // 4) * 4
        for d, lo in zip(DILS, _KLO)]

_CACHE: dict = {}

# tuning knobs
_PJBUFS = 3
_SMBUFS = 9
_SMVBUFS = 8
_LGBUFS = 2
_STBUFS = 1
_PSATBUFS = 2
_ATPBUFS = 3
_NORMRECIP = True
_KTAILOR = True
_QKILV = False
_STATSHARE = False
_STAGEOFF = (6, 4, 3)
_WARMUP = 20
_TAILQ4 = False
_POOLTAIL = False
_DEFER_OU1 = False
_DEFER_FROM = 4  # last tile: run ou1 matmuls after the stream so half0's store hides under them
_VQ4 = True      # v projection in column quarters for earlier start
_MASK8 = True    # fp8e5 DoubleRow mask matmul (0.5 cyc/row)
_DMATP = False   # XBAR DMA transpose for scores (loses: HWDGE overhead)


def _build_nc(reps=1, with_bias=False, f32r=True):
    from contextlib import ExitStack

    import concourse.bacc as bacc
    import concourse.tile as tile
    from concourse import mybir
    from concourse.masks import make_identity

    F32 = mybir.dt.float32
    FR = mybir.dt.float32r
    BF = mybir.dt.bfloat16
    AF = mybir.ActivationFunctionType

    nc = bacc.Bacc("TRN2", target_bir_lowering=False, debug=False, num_devices=1)

    xt_d = nc.dram_tensor("xt", [D, SPAN], FR, kind="ExternalInput").ap()
    xtb_d = nc.dram_tensor("xtb", [D, SPAN], BF, kind="ExternalInput").ap()
    wqr_d = nc.dram_tensor("wqr", [D, H * E], FR, kind="ExternalInput").ap()
    wkr_d = nc.dram_tensor("wkr", [D, H * E], FR, kind="ExternalInput").ap()
    wvr_d = nc.dram_tensor("wvr", [D, H * E], BF, kind="ExternalInput").ap()
    wos_d = nc.dram_tensor("wos", [H, E, D], BF, kind="ExternalInput").ap()
    mk_d = nc.dram_tensor("mk", [H, 128, 256], BF, kind="ExternalInput").ap()
    if _MASK8:
        F8 = mybir.dt.float8e5
        mk8_d = nc.dram_tensor("mk8", [128, 2 * H * 256], F8, kind="ExternalInput").ap()
        id8_d = nc.dram_tensor("id8", [128, 2 * 128], F8, kind="ExternalInput").ap()
    if with_bias:
        bqtk_d = nc.dram_tensor("bqtk", [E, 2 * H], F32, kind="ExternalInput").ap()
        bvb_d = nc.dram_tensor("bvb", [128, H * E], BF, kind="ExternalInput").ap()
        bob_d = nc.dram_tensor("bob", [128, D], F32, kind="ExternalInput").ap()
    out_d = nc.dram_tensor("out", [CHUNK, D], F32, kind="ExternalOutput").ap()
    import os as _os
    _dbg = bool(int(_os.environ.get("K_DBG", "0")))
    if _dbg:
        qdbg_d = nc.dram_tensor("qdbg", [128, H * CHUNK], FR, kind="ExternalOutput").ap()
        kdbg_d = nc.dram_tensor("kdbg", [128, H * SPAN], FR, kind="ExternalOutput").ap()
        vdbg_d = nc.dram_tensor("vdbg", [128, NST * H * E], BF, kind="ExternalOutput").ap()
        exdbg_d = nc.dram_tensor("exdbg", [128, 256], F32, kind="ExternalOutput").ap()
        scdbg_d = nc.dram_tensor("scdbg", [128, 256], BF, kind="ExternalOutput").ap()

    with tile.TileContext(nc) as tc, ExitStack() as ctx:
        const_p = ctx.enter_context(tc.tile_pool(name="const", bufs=1))
        big_p = ctx.enter_context(tc.tile_pool(name="big", bufs=1))
        sm_p = ctx.enter_context(tc.tile_pool(name="sm", bufs=_SMBUFS if not with_bias else 5))
        smv_p = ctx.enter_context(tc.tile_pool(name="smv", bufs=_SMVBUFS))
        at_p = ctx.enter_context(tc.tile_pool(name="atsb", bufs=_ATPBUFS))
        ob_p = ctx.enter_context(tc.tile_pool(name="ob", bufs=2))
        # pj tiles (phase 1) and ou tiles (phase 2) share one 2-bank pool
        ps_pj = ctx.enter_context(tc.tile_pool(name="ps_pj", bufs=_PJBUFS, space="PSUM"))
        ps_lg = ctx.enter_context(tc.tile_pool(name="ps_lg", bufs=_LGBUFS, space="PSUM"))
        if not _DMATP:
            ps_st = ctx.enter_context(tc.tile_pool(name="ps_st", bufs=_STBUFS, space="PSUM"))
        ps_at = ctx.enter_context(tc.tile_pool(name="ps_at", bufs=_PSATBUFS, space="PSUM"))
        if _QKILV:
            ps_ou = ctx.enter_context(tc.tile_pool(name="ps_ou", bufs=2, space="PSUM"))
        else:
            ps_ou = ps_pj

        # constants
        ident = const_p.tile([128, 128], BF)
        make_identity(nc, ident)
        if _WARMUP:
            # dummy matmuls ramp the PE clock to full speed before the first
            # real (DMA-gated) matmul arrives (~5us in)
            wrm = const_p.tile([128, 256], BF)
            nc.gpsimd.memzero(wrm)
            for _w in range(_WARMUP):
                wp = ps_lg.tile([128, 256], F32, tag="lg")
                nc.tensor.matmul(wp, wrm[:, 0:128], wrm, start=True, stop=True)

        for _rep in range(reps):
            # --- resident loads, ordered so the QK projection starts ASAP ---
            # v weights + bf16 x first (v projection runs first: its inputs
            # are cheap, so PE starts while the fat f32 q/k weights stream)
            xtb_g = []
            wv_q = []
            if _VQ4:
                # column-quarter Wv tiles: the first v pass needs only 1/4 of
                # Wv, so PE starts as soon as xtb + one quarter land
                wt = big_p.tile([128, DC, 256], BF, tag="wvq0")
                nc.sync.dma_start(wt, wvr_d[:, 0:256]
                                  .rearrange("(c p) n -> p c n", p=128))
                wv_q.append(wt)
            for x in range(4):
                xbg = big_p.tile([128, 2, SPAN], BF, tag=f"xtb{x}")
                nc.sync.dma_start(
                    xbg, xtb_d[256 * x:256 * (x + 1), :]
                    .rearrange("(c p) s -> p c s", p=128))
                xtb_g.append(xbg)
            if _VQ4:
                for qq in range(1, 4):
                    wt = big_p.tile([128, DC, 256], BF, tag=f"wvq{qq}")
                    nc.sync.dma_start(
                        wt, wvr_d[:, 256 * qq:256 * (qq + 1)]
                        .rearrange("(c p) n -> p c n", p=128))
                    wv_q.append(wt)
            else:
                wv_t = []
                for x in range(2):
                    wt = big_p.tile([128, 4, 512], BF, tag=f"wv0{x}")
                    nc.sync.dma_start(
                        wt, wvr_d[512 * x:512 * (x + 1), 0:512]
                        .rearrange("(c p) n -> p c n", p=128))
                    wv_t.append(wt)
                for x in range(2):
                    wt = big_p.tile([128, 4, 512], BF, tag=f"wv1{x}")
                    nc.sync.dma_start(
                        wt, wvr_d[512 * x:512 * (x + 1), 512:1024]
                        .rearrange("(c p) n -> p c n", p=128))
                    wv_t.append(wt)
                wv_c = [[wv_t[0][:, c, :] if c < 4 else wv_t[1][:, c - 4, :]
                         for c in range(DC)],
                        [wv_t[2][:, c, :] if c < 4 else wv_t[3][:, c - 4, :]
                         for c in range(DC)]]
            xtb_c = [xtb_g[c // 2][:, c % 2] for c in range(DC)]
            wq_g = []
            wk_g = []
            xt_g = []
            t = big_p.tile([128, DC, 2 * E], FR, tag="wq0")
            nc.sync.dma_start(t, wqr_d[:, 0:2 * E]
                              .rearrange("(c p) n -> p c n", p=128))
            wq_g.append(t)
            for x in range(4):
                xg = big_p.tile([128, 2, SPAN], FR, tag=f"xt{x}")
                nc.sync.dma_start(
                    xg, xt_d[256 * x:256 * (x + 1), :]
                    .rearrange("(c p) s -> p c s", p=128))
                xt_g.append(xg)
            t = big_p.tile([128, DC, 2 * E], FR, tag="wk0")
            nc.sync.dma_start(t, wkr_d[:, 0:2 * E]
                              .rearrange("(c p) n -> p c n", p=128))
            wk_g.append(t)
            xt_c = [xt_g[c // 2][:, c % 2] for c in range(DC)]
            if with_bias:
                bqtk_sb = big_p.tile([128, 2 * H], F32, tag="bqtk")
                nc.sync.dma_start(bqtk_sb, bqtk_d)
            for g in range(1, 4):
                t = big_p.tile([128, DC, 2 * E], FR, tag=f"wq{g}")
                nc.sync.dma_start(
                    t, wqr_d[:, 2 * E * g:2 * E * (g + 1)]
                    .rearrange("(c p) n -> p c n", p=128))
                wq_g.append(t)
                t = big_p.tile([128, DC, 2 * E], FR, tag=f"wk{g}")
                nc.sync.dma_start(
                    t, wkr_d[:, 2 * E * g:2 * E * (g + 1)]
                    .rearrange("(c p) n -> p c n", p=128))
                wk_g.append(t)
            if with_bias:
                bvb_sb = big_p.tile([128, H * E], BF, tag="bvb")
                nc.sync.dma_start(bvb_sb, bvb_d)
            if _MASK8:
                mk8_sb = big_p.tile([128, H, 2, 256], F8, tag="mk8")
                nc.sync.dma_start(mk8_sb, mk8_d.rearrange("p (h k t) -> p h k t", k=2, t=256))
                id8_sb = big_p.tile([128, 2, 128], F8, tag="id8")
                nc.sync.dma_start(id8_sb, id8_d.rearrange("p (k t) -> p k t", k=2))
            else:
                mk_sb = big_p.tile([128, H, 256], BF, tag="mk")
                nc.sync.dma_start(mk_sb, mk_d.rearrange("h p t -> p h t"))
            wos_sb = big_p.tile([128, H, D], BF, tag="wos")
            nc.sync.dma_start(wos_sb, wos_d.rearrange("h e d -> e h d"))
            if with_bias:
                bob_sb = big_p.tile([128, D], F32, tag="bob")
                nc.sync.dma_start(bob_sb, bob_d)

            # persistent projection outputs
            qT_sb = big_p.tile([128, H, CHUNK], FR, tag="qT")   # [e, h, s]
            kT_sb = big_p.tile([128, H, SPAN], FR, tag="kT")    # [e, h, s]
            v_sb = big_p.tile([128, NST, H * E], BF, tag="v")   # [s, tile, h*E+e]

            # k spans: head h with dilation d only ever reads kT columns
            # [64-off, 64-off+512+15d) of the 640 layout (off = d*(KW-1)//2);
            # skip projecting the permanently-masked edges, zero them once so
            # the dense logits window never reads garbage.
            if _KTAILOR:
                for h in range(H):
                    lo, w = _KLO[h], _KW_[h]
                    if lo > 0:
                        nc.gpsimd.memzero(kT_sb[:, h, 0:lo])
                    if lo + w < SPAN:
                        nc.gpsimd.memzero(kT_sb[:, h, lo + w:SPAN])

            # ---- phase 1b first: v projection (bf16 x/Wv, cheap inputs) ----
            # quarter-major so Wv quarter q isn't needed until q/4 through
            if _VQ4:
                for qq in range(4):
                    for j in range(NST):
                        vp = ps_pj.tile([128, 512], F32, tag="pj")
                        nsl = slice(256 * qq, 256 * (qq + 1))
                        for c in range(DC):
                            nc.tensor.matmul(vp[:, 0:256],
                                             xtb_c[c][:, 128 * j:128 * (j + 1)],
                                             wv_q[qq][:, c, :], start=(c == 0),
                                             stop=(c == DC - 1))
                        if with_bias:
                            nc.vector.tensor_add(v_sb[:, j, nsl], vp[:, 0:256],
                                                 bvb_sb[:, nsl])
                        else:
                            nc.scalar.copy(v_sb[:, j, nsl], vp[:, 0:256])
            else:
                for half in range(2):
                    for j in range(NST):
                        vp = ps_pj.tile([128, 512], F32, tag="pj")
                        nsl = slice(512 * half, 512 * (half + 1))
                        for c in range(DC):
                            nc.tensor.matmul(vp, xtb_c[c][:, 128 * j:128 * (j + 1)],
                                             wv_c[half][c], start=(c == 0),
                                             stop=(c == DC - 1))
                        if with_bias:
                            nc.vector.tensor_add(v_sb[:, j, nsl], vp, bvb_sb[:, nsl])
                        else:
                            nc.scalar.copy(v_sb[:, j, nsl], vp)

            # ---- phase 1a: q/k projections per head (W chunks stationary) ----
            def QK(h):
                g, col = h // 2, (h % 2) * E
                qp = ps_pj.tile([128, 512], F32, tag="pj")
                for c in range(DC):
                    nc.tensor.matmul(qp, wq_g[g][:, c, col:col + E],
                                     xt_c[c][:, HALO:HALO + CHUNK],
                                     start=(c == 0), stop=(c == DC - 1))
                if with_bias:
                    nc.scalar.activation(qT_sb[:, h, :], qp, AF.Identity,
                                         bias=bqtk_sb[:, h:h + 1], scale=1.0)
                else:
                    nc.scalar.copy(qT_sb[:, h, :], qp)
                lo, w = (_KLO[h], _KW_[h]) if _KTAILOR else (0, SPAN)
                w1 = (w + 1) // 2
                for sl in (slice(lo, lo + w1), slice(lo + w1, lo + w)):
                    n = sl.stop - sl.start
                    kp = ps_pj.tile([128, 512], F32, tag="pj")
                    for c in range(DC):
                        nc.tensor.matmul(kp[:, 0:n], wk_g[g][:, c, col:col + E],
                                         xt_c[c][:, sl],
                                         start=(c == 0), stop=(c == DC - 1))
                    if with_bias:
                        nc.scalar.activation(kT_sb[:, h, sl], kp[:, 0:n],
                                             AF.Identity,
                                             bias=bqtk_sb[:, H + h:H + h + 1],
                                             scale=1.0)
                    else:
                        nc.scalar.copy(kT_sb[:, h, sl], kp[:, 0:n])

            for h in range(2 if _QKILV else H - 2):
                QK(h)

            if _dbg:
                nc.sync.dma_start(qdbg_d, qT_sb.rearrange("p h s -> p (h s)"))
                nc.sync.dma_start(kdbg_d, kT_sb.rearrange("p h s -> p (h s)"))
                nc.sync.dma_start(vdbg_d, v_sb.rearrange("p t n -> p (t n)"))

            # ---- phase 2: attention + output projection ----
            # One flat 32-chain software pipeline across all (i, h): F(c) =
            # mask+logits matmuls + exp (ACT) + normalize (Pool), TA(c) =
            # score transposes + attnT matmuls, O(c) = out-proj accumulate.
            # F runs 2 chains ahead and TA 1 chain ahead of O, so PE never
            # waits for the softmax round-trip, including across i-boundaries.
            scs = {}
            atss = {}
            ous = {}

            def F(c):
                i, h = divmod(c, H)
                lg = ps_lg.tile([128, 256], F32, tag="lg")
                if _MASK8:
                    nc.tensor.matmul(lg, id8_sb, mk8_sb[:, h],
                                     start=True, stop=False,
                                     perf_mode=mybir.MatmulPerfMode.DoubleRow)
                else:
                    nc.tensor.matmul(lg, ident, mk_sb[:, h, :],
                                     start=True, stop=False)
                nc.tensor.matmul(lg, qT_sb[:, h, 128 * i:128 * (i + 1)],
                                 kT_sb[:, h, 128 * i:128 * i + 256],
                                 start=False, stop=True)
                ex = sm_p.tile([128, 256], F32, tag="ex")
                se = smv_p.tile([128, 1], F32, tag="se")
                nc.scalar.activation(ex, lg, AF.Exp, bias=0.0, scale=1.0,
                                     accum_out=se)
                if _dbg and c == 0:
                    nc.sync.dma_start(exdbg_d, ex)
                sc = sm_p.tile([128, 256], BF, tag="sc")
                if _NORMRECIP:
                    nc.gpsimd.normalize_recip(sc, ex, se)
                else:
                    rc = smv_p.tile([128, 1], F32, tag="rc")
                    nc.vector.reciprocal(rc, se)
                    nc.vector.tensor_scalar_mul(sc, ex, rc)
                if _dbg and c == 0:
                    nc.sync.dma_start(scdbg_d, sc)
                scs[c] = sc

            scts = {}

            def T(c):
                sc = scs.pop(c)
                sct = sm_p.tile([128, 256], BF, tag="sct")
                if _DMATP:
                    # XBAR DMA transpose: [128,256] -> two stacked [128,128]
                    # transposed chunks, replacing 2 PE transposes + PSUM +
                    # the DVE copy
                    nc.sync.dma_start_transpose(
                        sct.rearrange("p (j t) -> p j t", j=2), sc)
                else:
                    st = ps_st.tile([128, 256], BF, tag="st")
                    nc.tensor.transpose(st[:, 0:128], sc[:, 0:128], ident)
                    nc.tensor.transpose(st[:, 128:256], sc[:, 128:256], ident)
                    nc.vector.tensor_copy(sct, st)
                scts[c] = sct

            def A(c):
                i, h = divmod(c, H)
                sct = scts.pop(c)
                at = ps_at.tile([128, 128], F32, tag="at")
                nc.tensor.matmul(at, v_sb[:, i, E * h:E * (h + 1)],
                                 sct[:, 0:128], start=True, stop=False)
                nc.tensor.matmul(at, v_sb[:, i + 1, E * h:E * (h + 1)],
                                 sct[:, 128:256], start=False, stop=True)
                ats = at_p.tile([128, 128], BF, tag="ats")
                nc.vector.tensor_copy(ats, at)
                atss[c] = ats

            NCH = NQT * H
            FD, TD, AD = _STAGEOFF
            # prologue rides ahead of the last two QK heads to hide fill
            for c in range(2):
                F(c)
            if not _QKILV:
                QK(H - 2)
            for c in range(2, FD):
                F(c)
            for c in range(TD):
                T(c)
            for c in range(AD):
                A(c)
            if not _QKILV:
                QK(H - 1)
            for c in range(NCH):
                i, h = divmod(c, H)
                if _QKILV and 2 <= c + FD < H:
                    QK(c + FD)
                if c + FD < NCH:
                    F(c + FD)
                if c + TD < NCH:
                    T(c + TD)
                if c + AD < NCH:
                    A(c + AD)
                if h == 0:
                    ou0 = ps_ou.tile([128, 512], F32, tag="pj")
                    ou1 = ps_ou.tile([128, 512], F32, tag="pj")
                    ous[i] = (ou0, ou1)
                ou0, ou1 = ous[i]
                last_i = _DEFER_OU1 and i == NQT - 1
                defer = last_i and h >= _DEFER_FROM
                ats = atss.pop(c)
                nc.tensor.matmul(ou0, ats, wos_sb[:, h, 0:512],
                                 start=(h == 0), stop=(h == H - 1))
                if defer:
                    atss[("keep", h)] = ats
                else:
                    nc.tensor.matmul(ou1, ats, wos_sb[:, h, 512:1024],
                                     start=(h == 0),
                                     stop=(h == H - 1 and not last_i))
                if h == H - 1 and not last_i:
                    # quarter-copies on ACT+DVE in parallel so the ou banks
                    # free fast; each out piece DMAs as soon as it lands (the
                    # last tile fires per-quarter DMAs to shorten the tail)
                    ob = ob_p.tile([128, D], F32, tag="ob")
                    per_q = _TAILQ4 and i == NQT - 1
                    for q, (ou, qsl, osl) in enumerate((
                            (ou0, slice(0, 256), slice(0, 256)),
                            (ou0, slice(256, 512), slice(256, 512)),
                            (ou1, slice(0, 256), slice(512, 768)),
                            (ou1, slice(256, 512), slice(768, 1024)))):
                        if with_bias:
                            eng = nc.vector.tensor_add
                            args = (ob[:, osl], ou[:, qsl], bob_sb[:, osl])
                        elif q % 2 == 0:
                            eng = nc.scalar.copy
                            args = (ob[:, osl], ou[:, qsl])
                        else:
                            eng = nc.vector.tensor_copy
                            args = (ob[:, osl], ou[:, qsl])
                        eng(*args)
                        dmae = nc.gpsimd if (_POOLTAIL and i == NQT - 1) else nc.sync
                        if per_q:
                            dmae.dma_start(
                                out_d[128 * i:128 * (i + 1), osl], ob[:, osl])
                        elif q % 2 == 1:
                            dmae.dma_start(
                                out_d[128 * i:128 * (i + 1), osl.start - 256:
                                      osl.stop], ob[:, osl.start - 256:osl.stop])

            if _DEFER_OU1:
                # last tile: half0 stores while the deferred ou1 accumulates
                li = NQT - 1
                ou0, ou1 = ous[li]
                obl = ob_p.tile([128, D], F32, tag="ob")
                nc.scalar.copy(obl[:, 0:256], ou0[:, 0:256])
                nc.vector.tensor_copy(obl[:, 256:512], ou0[:, 256:512])
                nc.sync.dma_start(out_d[128 * li:128 * (li + 1), 0:512],
                                  obl[:, 0:512])
                for h in range(_DEFER_FROM, H):
                    ats = atss.pop(("keep", h))
                    nc.tensor.matmul(ou1, ats, wos_sb[:, h, 512:1024],
                                     start=False, stop=(h == H - 1))
                nc.scalar.copy(obl[:, 512:768], ou1[:, 0:256])
                nc.vector.tensor_copy(obl[:, 768:1024], ou1[:, 256:512])
                nc.sync.dma_start(out_d[128 * li:128 * (li + 1), 512:1024],
                                  obl[:, 512:1024])

    nc.compile()
    return nc


def _host_prep(x, Wq, bq, Wk, bk, Wv, bv, Wo, bo, dilations, with_bias):
    import ml_dtypes
    f = np.float32
    bf = ml_dtypes.bfloat16
    x = np.asarray(x, f)
    x_pad = np.zeros((B, S + 2 * HALO, D), f)
    x_pad[:, HALO:HALO + S] = x

    def packw(W):  # [H, D, E] -> [D, H*E] f32 (fed to f32r matmuls)
        return np.ascontiguousarray(
            np.asarray(W, f).transpose(1, 0, 2).reshape(D, H * E))

    wos = np.ascontiguousarray(
        np.asarray(Wo, f) * np.float32(E) ** f(-0.5)).astype(bf)

    dil = np.asarray(dilations).astype(np.int64)
    assert list(dil) == DILS, f"unexpected dilations {dil}"
    masks = np.full((H, 128, 256), MASKVAL, f)
    s_i = np.arange(128)[:, None]
    t_i = np.arange(256)[None, :]
    for h in range(H):
        d = int(dil[h])
        off = (d * (KW - 1)) // 2
        delta = t_i - s_i - HALO + off
        win = (delta >= 0) & (delta <= (KW - 1) * d) & (delta % d == 0)
        masks[h][win] = 0.0

    shared = {
        "wqr": packw(Wq), "wkr": packw(Wk),
        "wvr": packw(Wv).astype(bf),
        "wos": wos, "mk": masks.astype(bf),
    }
    f8 = ml_dtypes.float8_e5m2
    mk8 = np.zeros((128, H, 2, 256), f)
    mk8[:, :, 0, :] = np.float32(-28672.0) * (masks.transpose(1, 0, 2) != 0)
    shared["mk8"] = np.ascontiguousarray(mk8.reshape(128, -1)).astype(f8)
    id8 = np.zeros((128, 2, 128), f)
    id8[:, 0, :] = np.eye(128, dtype=f)
    shared["id8"] = np.ascontiguousarray(id8.reshape(128, -1)).astype(f8)
    if with_bias:
        shared["bqtk"] = np.ascontiguousarray(
            np.concatenate([np.asarray(bq, f).T, np.asarray(bk, f).T], axis=1))
        shared["bvb"] = np.ascontiguousarray(np.broadcast_to(
            np.asarray(bv, f).reshape(1, H * E), (128, H * E)).astype(bf))
        shared["bob"] = np.ascontiguousarray(
            np.broadcast_to(np.asarray(bo, f).reshape(1, D), (128, D)))
    in_maps = []
    for c in range(NC_):
        b, idx = divmod(c, 4)
        xt = np.ascontiguousarray(
            x_pad[b, idx * CHUNK: idx * CHUNK + SPAN].T)
        in_maps.append({"xt": xt, "xtb": xt.astype(bf), **shared})
    return in_maps


def kernel(x, Wq, bq, Wk, bk, Wv, bv, Wo, bo, dilations):
    from concourse.bass_utils import run_bass_kernel_spmd

    with_bias = bool(np.any(bq) or np.any(bk) or np.any(bv) or np.any(bo))
    key = f"nc{int(with_bias)}"
    if key not in _CACHE:
        _CACHE[key] = _build_nc(with_bias=with_bias)
    nc = _CACHE[key]

    in_maps = _host_prep(x, Wq, bq, Wk, bk, Wv, bv, Wo, bo, dilations,
                         with_bias)
    res = run_bass_kernel_spmd(nc, in_maps, core_ids=list(range(NC_)))

    out = np.empty((B, S, D), np.float32)
    for c in range(NC_):
        b, idx = divmod(c, 4)
        out[b, idx * CHUNK:(idx + 1) * CHUNK] = res.results[c]["out"]
    return out
